# revision 2
# baseline (speedup 1.0000x reference)
"""Multi-head attention (N=4, S=2048, EMB=1024, 16 heads) on 8 Trainium2 cores.

Sharding: 4 batches x 2 head-groups (8 heads each). Each core computes its
batch's Q/K/V projections for its 8 heads, full softmax attention, and a
partial out-projection; the host sums the two head-group partials per batch.

Math notes (all exact w.r.t. the reference):
  - bk drops out of softmax (adds a per-query constant to every logit).
  - bq is folded into the Q^T eviction on-device (per-partition scalar add).
  - bv contributes A@1 * bv^T = bv^T per token (softmax rows sum to 1), so
    host adds (bv @ Wo + bo) once at the end.
Scores are small (std ~0.25) so softmax runs without max-subtraction.

Device data layout is feature-major ("transposed"): host supplies a^T so all
matmul contractions have their index on SBUF partitions. The softmax
denominator comes free from a ones-column appended to V in the A.V matmul;
1/Z is broadcast across partitions with a K=1 matmul against a ones row.
"""

import numpy as np
from contextlib import ExitStack

EMB = 1024
NH_LOCAL = 8  # heads per core
NPAIR = 4  # head pairs per core (2 heads packed per 128 partitions)
S_FULL = 2048
NCORES = 8
SCALE = float(np.sqrt(np.float32(EMB)))  # 32.0


def build_nc(S=S_FULL):
    import concourse.bacc as bacc
    import concourse.tile as tile
    from concourse import mybir

    bf = mybir.dt.bfloat16
    f32 = mybir.dt.float32
    Act = mybir.ActivationFunctionType

    F = 512  # local features (8 heads x 64)
    EC = EMB // 128  # emb chunks
    T16 = S // 128  # 128-token chunks
    TB = S // 512  # 512-token blocks
    KC = S // 128  # 128-key chunks
    KCG = KC // 2

    nc = bacc.Bacc("TRN2", target_bir_lowering=False, debug=False)

    aT_d = nc.dram_tensor("aT", [EC, 128, S], bf, kind="ExternalInput")
    wq_d = nc.dram_tensor("wq", [EC, 128, F], bf, kind="ExternalInput")
    wk_d = nc.dram_tensor("wk", [EC, 128, F], bf, kind="ExternalInput")
    wv_d = nc.dram_tensor("wv", [EC, 128, F], bf, kind="ExternalInput")
    wo_d = nc.dram_tensor("wo", [NPAIR, 128, EMB], bf, kind="ExternalInput")
    bq_d = nc.dram_tensor("bq", [128, NPAIR], f32, kind="ExternalInput")
    out_d = nc.dram_tensor("out", [S, EMB], f32, kind="ExternalOutput")

    with ExitStack() as top:
        tc = top.enter_context(tile.TileContext(nc))
        const = top.enter_context(tc.tile_pool(name="const", bufs=1))

        wo_sb = const.tile([128, NPAIR, EMB], bf)
        bq_sb = const.tile([128, NPAIR], f32)
        ones_sb = const.tile([128, 64], f32)
        qt_sb = const.tile([128, NPAIR, S], bf)
        kt_sb = const.tile([128, NPAIR, S], bf)
        v_sb = const.tile([128, T16, NH_LOCAL, 65], bf)
        ctxT = const.tile([128, NPAIR, S], bf)

        for p in range(NPAIR):
            nc.sync.dma_start(wo_sb[:, p, :], wo_d[p])
        nc.sync.dma_start(bq_sb[:], bq_d[:])
        nc.vector.memset(ones_sb[:], 1.0)
        nc.vector.memset(v_sb[:, :, :, 64:65], 1.0)

        # ---- Phase 1: Q^T / K^T / V projections ----
        with tc.tile_pool(name="early", bufs=1) as early:
            aT = early.tile([128, EC, S], bf)
            wq_sb = early.tile([128, EC, F], bf)
            wk_sb = early.tile([128, EC, F], bf)
            wv_sb = early.tile([128, EC, F], bf)
            for c in range(EC):
                nc.sync.dma_start(aT[:, c, :], aT_d[c])
                nc.sync.dma_start(wq_sb[:, c, :], wq_d[c])
                nc.sync.dma_start(wk_sb[:, c, :], wk_d[c])
                nc.sync.dma_start(wv_sb[:, c, :], wv_d[c])

            with tc.tile_pool(name="qkvps", bufs=1, space="PSUM") as qkvps:
                for p in range(NPAIR):
                    for tb in range(TB):
                        ts = slice(tb * 512, (tb + 1) * 512)
                        ps_q = qkvps.tile([128, 512], f32, tag="ps", bufs=6)
                        for c in range(EC):
                            nc.tensor.matmul(
                                ps_q[:],
                                wq_sb[:, c, p * 128 : (p + 1) * 128],
                                aT[:, c, ts],
                                start=(c == 0),
                                stop=(c == EC - 1),
                            )
                        nc.vector.tensor_scalar_add(
                            qt_sb[:, p, ts], ps_q[:], bq_sb[:, p : p + 1]
                        )
                        ps_k = qkvps.tile([128, 512], f32, tag="ps", bufs=6)
                        for c in range(EC):
                            nc.tensor.matmul(
                                ps_k[:],
                                wk_sb[:, c, p * 128 : (p + 1) * 128],
                                aT[:, c, ts],
                                start=(c == 0),
                                stop=(c == EC - 1),
                            )
                        nc.vector.tensor_copy(kt_sb[:, p, ts], ps_k[:])
                for t in range(T16):
                    ps_v = qkvps.tile([128, 512], f32, tag="ps", bufs=6)
                    for c in range(EC):
                        nc.tensor.matmul(
                            ps_v[:],
                            aT[:, c, t * 128 : (t + 1) * 128],
                            wv_sb[:, c, :],
                            start=(c == 0),
                            stop=(c == EC - 1),
                        )
                    nc.vector.tensor_copy(
                        v_sb[:, t, :, 0:64],
                        ps_v[:].rearrange("p (h d) -> p h d", h=NH_LOCAL),
                    )

        # ---- Phase 2: attention (scores^T -> exp -> A.V -> normalize) ----
        with (
            tc.tile_pool(name="attn", bufs=1) as attn,
            tc.tile_pool(name="scps", bufs=1, space="PSUM") as scps,
            tc.tile_pool(name="avps", bufs=1, space="PSUM") as avps,
            tc.tile_pool(name="rbps", bufs=1, space="PSUM") as rbps,
        ):
            for p in range(NPAIR):
                for tb in range(TB):
                    ts = slice(tb * 512, (tb + 1) * 512)
                    expsA = attn.tile([128, KC, 512], bf, tag="exps", bufs=3)
                    expsB = attn.tile([128, KC, 512], bf, tag="exps", bufs=3)
                    for g in range(KCG):
                        scA = scps.tile([128, 2, 512], f32, tag="sc", bufs=2)
                        scB = scps.tile([128, 2, 512], f32, tag="sc", bufs=2)
                        for j in range(2):
                            kc = 2 * g + j
                            ks = slice(kc * 128, (kc + 1) * 128)
                            nc.tensor.matmul(
                                scA[:, j, :],
                                kt_sb[0:64, p, ks],
                                qt_sb[0:64, p, ts],
                                start=True,
                                stop=True,
                            )
                            nc.tensor.matmul(
                                scB[:, j, :],
                                kt_sb[64:128, p, ks],
                                qt_sb[64:128, p, ts],
                                start=True,
                                stop=True,
                            )
                        nc.scalar.activation(
                            expsA[:, 2 * g : 2 * g + 2, :],
                            scA[:],
                            Act.Exp,
                            scale=1.0 / SCALE,
                        )
                        nc.scalar.activation(
                            expsB[:, 2 * g : 2 * g + 2, :],
                            scB[:],
                            Act.Exp,
                            scale=1.0 / SCALE,
                        )
                    for hh, exps in ((0, expsA), (1, expsB)):
                        h = 2 * p + hh
                        av = avps.tile([65, 512], f32, tag="av", bufs=2)
                        for kc in range(KC):
                            nc.tensor.matmul(
                                av[:],
                                v_sb[:, kc, h, :],
                                exps[:, kc, :],
                                start=(kc == 0),
                                stop=(kc == KC - 1),
                            )
                        z_sb = attn.tile([65, 512], f32, tag="z", bufs=2)
                        nc.vector.tensor_copy(z_sb[64:65, :], av[64:65, :])
                        r_sb = attn.tile([65, 512], f32, tag="r", bufs=2)
                        nc.vector.reciprocal(r_sb[64:65, :], z_sb[64:65, :])
                        rb = rbps.tile([64, 512], f32, tag="rb", bufs=2)
                        nc.tensor.matmul(
                            rb[:],
                            ones_sb[64:65, :],
                            r_sb[64:65, :],
                            start=True,
                            stop=True,
                        )
                        rb_sb = attn.tile([64, 512], f32, tag="rbs", bufs=2)
                        nc.vector.tensor_copy(rb_sb[:], rb[:])
                        nc.vector.tensor_mul(
                            ctxT[hh * 64 : (hh + 1) * 64, p, ts],
                            av[0:64, :],
                            rb_sb[:],
                        )

        # ---- Phase 3: out-projection (partial; host sums head-groups) ----
        with (
            tc.tile_pool(name="ostage", bufs=1) as ostage,
            tc.tile_pool(name="ops", bufs=1, space="PSUM") as opsp,
        ):
            for t in range(T16):
                for eb in range(2):
                    po = opsp.tile([128, 512], f32, tag="po", bufs=6)
                    for p in range(NPAIR):
                        nc.tensor.matmul(
                            po[:],
                            ctxT[:, p, t * 128 : (t + 1) * 128],
                            wo_sb[:, p, eb * 512 : (eb + 1) * 512],
                            start=(p == 0),
                            stop=(p == NPAIR - 1),
                        )
                    o_sb = ostage.tile([128, 512], f32, tag="ost", bufs=4)
                    nc.vector.tensor_copy(o_sb[:], po[:])
                    nc.sync.dma_start(
                        out_d[t * 128 : (t + 1) * 128, eb * 512 : (eb + 1) * 512],
                        o_sb[:],
                    )

    return nc


_cache = {}


def _get_built():
    if "nc" not in _cache:
        nc = build_nc(S_FULL)
        nc.compile()
        _cache["nc"] = nc
    return _cache["nc"]


def shard_inputs(a, Wq, bq, Wk, Wv, Wo, S=S_FULL):
    import ml_dtypes

    bfnp = ml_dtypes.bfloat16
    in_maps = []
    for c in range(NCORES):
        b, hg = c // 2, c % 2
        sl = slice(hg * 512, (hg + 1) * 512)
        aT = np.ascontiguousarray(a[b].T).reshape(8, 128, S).astype(bfnp)
        wq_c = np.ascontiguousarray(Wq[:, sl]).reshape(8, 128, 512).astype(bfnp)
        wk_c = np.ascontiguousarray(Wk[:, sl]).reshape(8, 128, 512).astype(bfnp)
        wv_c = np.ascontiguousarray(Wv[:, sl]).reshape(8, 128, 512).astype(bfnp)
        wo_c = np.ascontiguousarray(Wo[sl, :]).reshape(4, 128, EMB).astype(bfnp)
        bq_c = np.ascontiguousarray(bq[sl].reshape(4, 128).T).astype(np.float32)
        in_maps.append(
            {"aT": aT, "wq": wq_c, "wk": wk_c, "wv": wv_c, "wo": wo_c, "bq": bq_c}
        )
    return in_maps


def kernel(a, Wq, bq, Wk, bk, Wv, bv, Wo, bo, trace=False):
    from concourse.bass_utils import run_bass_kernel_spmd

    a = np.asarray(a, np.float32)
    Wq = np.asarray(Wq, np.float32)
    bq = np.asarray(bq, np.float32)
    Wk = np.asarray(Wk, np.float32)
    Wv = np.asarray(Wv, np.float32)
    bv = np.asarray(bv, np.float32)
    Wo = np.asarray(Wo, np.float32)
    bo = np.asarray(bo, np.float32)

    nc = _get_built()
    in_maps = shard_inputs(a, Wq, bq, Wk, Wv, Wo)
    res = run_bass_kernel_spmd(nc, in_maps, list(range(NCORES)), trace=trace)
    _cache["last_result"] = res

    corr = (bo + bv @ Wo).astype(np.float32)
    out = np.empty((a.shape[0], S_FULL, EMB), np.float32)
    for b in range(a.shape[0]):
        out[b] = res.results[2 * b]["out"] + res.results[2 * b + 1]["out"] + corr[None, :]
    return out
